# revision 19
# baseline (speedup 1.0000x reference)
"""Trainium2 Bass kernel for the cosine-gated LSTM cell (CGLSTMCellv1).

Full inputs in, full outputs out. Data-parallel: batch sharded across 8
NeuronCores, weights replicated, no cross-core comms.

Key performance choices vs the fp32 baseline:
  - All matmuls in bf16 (fp32 matmul runs at 4 cycles/row on TRN2 PE; bf16
    at 1). Operands are cast and laid out host-side.
  - x^T / hx^T are packed on the host into the exact SBUF tile layout
    [p, t, kb, r], eliminating all 128 PE transposes + PSUM round-trips.
  - W is host-packed per column-chunk [p, c, kb, n] so each chunk load is
    one contiguous 16KB-per-partition DMA; chunks are triple-buffered.
  - DMAs are issued in first-use order (Wm + x^T tiles first) so the first
    matmul starts ~10us in, not after all input DMAs.
  - LayerNorm rstd and the two cosine denominators use a DVE-only
    Newton-iteration rsqrt (bit hack + 2 iterations) instead of ACT Sqrt,
    so the scalar engine needs exactly one activation-table set
    (sigmoid_and_others: sigmoid/tanh/square/copy) -> one table load total.
  - Gate bias b is added during the PSUM->SBUF combine pass (free).
  - LN mean comes free from the combine pass accum; E[z^2] from an ACT
    Square accum pass; stats are batched per half-wave (4 tiles) on [P,4]
    tiles to shorten the stats barrier.
  - The o-gate's normalize/affine work runs on DVE (not Pool) because it
    sits in the post-matmul tail where Pool's 2.3us/op latency would be
    exposed; the earlier gates' affine runs on Pool to offload DVE while
    the PE still has matmul work to overlap.

Math per core (rows = local batch slice):
  mapped = x @ Wm + bm
  attn   = sigmoid(cos_sim(mapped, hx));  s = 1 + attn
  gates  = concat(s*x, hx) @ W + b   (s folded into x^T)
  i,f,g,o = LN-gates -> sigmoid/tanh
  cx_new = f*cx + i*g ; hx_new = o*tanh(cx_new)
  hx_mod = hx_new * (1 + sigmoid((cos_sim(hx_new,cx_new)+1)/2))
"""

import numpy as np

B_FULL, DIM_I, DIM_H = 8192, 1024, 1024
NCORES = 8
BL = B_FULL // NCORES  # 1024 rows per core
P = 128
H4 = 4 * DIM_H
NBT = BL // P               # 8 row tiles of 128
NKB1 = DIM_I // P           # 8  k-blocks for the x part
NKB2 = (DIM_I + DIM_H) // P  # 16 k-blocks for mm2
CHUNK = 512                 # W column chunk (= one PSUM bank of fp32)
NCHUNK = H4 // CHUNK        # 8 chunks total, 2 per gate
LN_EPS = 1e-5
COS_EPS2 = 1e-12
MAGIC = 0x5F3759DF

_cache = {}


def build_nc(nbt=NBT):
    from contextlib import ExitStack

    import concourse.bass as bass
    import concourse.mybir as mybir
    import concourse.tile as tile

    fp32 = mybir.dt.float32
    bf16 = mybir.dt.bfloat16
    i32 = mybir.dt.int32
    AF = mybir.ActivationFunctionType
    OP = mybir.AluOpType
    bl = nbt * P

    nc = bass.Bass()
    # host-packed inputs (see kernel() for the packing)
    xtd = nc.dram_tensor("xt4", [P, nbt, NKB1, P], bf16, kind="ExternalInput")
    htd = nc.dram_tensor("ht4", [P, nbt, NKB1, P], bf16, kind="ExternalInput")
    hxd = nc.dram_tensor("hxr", [bl, DIM_H], bf16, kind="ExternalInput")
    cxd = nc.dram_tensor("cxr", [bl, DIM_H], fp32, kind="ExternalInput")
    wpd = nc.dram_tensor("wp4", [P, NCHUNK, NKB2, CHUNK], bf16, kind="ExternalInput")
    wmd = nc.dram_tensor("wmp", [P, NKB1, DIM_H], bf16, kind="ExternalInput")
    bd = nc.dram_tensor("bb", [H4], bf16, kind="ExternalInput")
    bmd = nc.dram_tensor("bmb", [DIM_H], bf16, kind="ExternalInput")
    gd = nc.dram_tensor("gmb", [4, DIM_H], bf16, kind="ExternalInput")
    btd = nc.dram_tensor("btb", [4, DIM_H], bf16, kind="ExternalInput")
    hxo = nc.dram_tensor("hx_out", [bl, DIM_H], fp32, kind="ExternalOutput")
    cxo = nc.dram_tensor("cx_out", [bl, DIM_H], fp32, kind="ExternalOutput")

    def bcast_row(src_ap):
        # view an [N]-shaped AP as [P, N] with 0-step partition broadcast
        return bass.AP(
            tensor=src_ap.tensor, offset=src_ap.offset, ap=[[0, P]] + list(src_ap.ap)
        )

    with tile.TileContext(nc) as tc, ExitStack() as ctx:
        singles = ctx.enter_context(tc.tile_pool(name="singles", bufs=1))
        sm_pool = ctx.enter_context(tc.tile_pool(name="smalls", bufs=4))

        ident = singles.tile([P, P], fp32)
        from concourse.masks import make_identity

        make_identity(nc, ident)
        ones128 = singles.tile([P, P], fp32)
        nc.vector.memset(ones128, 1.0)
        zrow = singles.tile([P, P], fp32)
        nc.vector.memset(zrow, 0.0)
        magic_t = singles.tile([P, 8], i32)
        nc.vector.memset(magic_t, MAGIC)
        halfc = singles.tile([P, 1], fp32)
        nc.vector.memset(halfc, 0.5)
        junk = singles.tile([P, 1], fp32)
        nc.vector.memset(junk, 0.5)
        # prime the ACT table with the one set the whole kernel uses
        nc.scalar.activation(junk, junk, AF.Sigmoid)

        def newton_rsqrt(x_f32, n, name):
            """y ~= 1/sqrt(x) on DVE only: bit hack + 2 Newton steps."""
            xi = x_f32.bitcast(i32)
            sh = sm_pool.tile([P, n], i32, tag="nr_sh", name=f"sh_{name}")
            nc.vector.tensor_scalar(sh, xi, 1, None, op0=OP.logical_shift_right)
            yi = sm_pool.tile([P, n], i32, tag="nr_yi", name=f"yi_{name}")
            nc.vector.tensor_tensor(yi, magic_t[:, 0:n], sh, OP.subtract)
            y = yi.bitcast(fp32)
            for it in range(2):
                a = sm_pool.tile([P, n], fp32, tag="nr_a", name=f"a{it}_{name}")
                nc.vector.tensor_tensor(a, y, y, OP.mult)
                nc.vector.tensor_tensor(a, a, x_f32, OP.mult)
                nc.vector.tensor_scalar(a, a, -0.5, 1.5, OP.mult, OP.add)
                yn = sm_pool.tile([P, n], fp32, tag="nr_y", name=f"y{it}_{name}")
                nc.vector.tensor_tensor(yn, y, a, OP.mult)
                y = yn
            return y

        # persistent transposed activations (xsT scaled in place after mm1)
        xsT = singles.tile([P, nbt, NKB1, P], bf16)
        hxT = singles.tile([P, nbt, NKB1, P], bf16)
        bm_rep = singles.tile([P, DIM_H], bf16)
        b_rep = singles.tile([P, H4], bf16)

        # cos-gate scratch, batched across tiles as [P, nbt] columns
        dot_a = sm_pool.tile([P, nbt], fp32, tag="dot_a")
        sqm_a = sm_pool.tile([P, nbt], fp32, tag="sqm_a")
        sqh_a = sm_pool.tile([P, nbt], fp32, tag="sqh_a")
        s_a = sm_pool.tile([P, nbt], fp32, tag="s_a")

        iact_pool = ctx.enter_context(tc.tile_pool(name="iact", bufs=nbt))
        iact = [
            iact_pool.tile([P, DIM_H], fp32, tag="iact", name=f"iact{t}")
            for t in range(nbt)
        ]
        w_pool = ctx.enter_context(tc.tile_pool(name="wch", bufs=3))
        dump_pool = ctx.enter_context(tc.tile_pool(name="dump", bufs=2))
        wch = {}

        def load_wch(cid):
            if cid in wch or cid >= NCHUNK:
                return
            wch[cid] = w_pool.tile(
                [P, NKB2, CHUNK], bf16, tag="wch", name=f"wch{cid}"
            )
            nc.sync.dma_start(out=wch[cid], in_=wpd[:, cid])

        # ---------------- phase 1: mm1 + cosine gate ----------------
        with ExitStack() as p1:
            wm_pool = p1.enter_context(tc.tile_pool(name="wm", bufs=1))
            hx_pool = p1.enter_context(tc.tile_pool(name="hxr", bufs=1))
            map_pool = p1.enter_context(tc.tile_pool(name="map", bufs=3))
            ps_m1 = p1.enter_context(tc.tile_pool(name="psm1", bufs=2, space="PSUM"))
            ps_rep = p1.enter_context(tc.tile_pool(name="psrep", bufs=2, space="PSUM"))

            # DMAs in first-use order: Wm and x^T tiles gate the first
            # matmuls; W chunk 0/1 must land before the first wave (~37us),
            # so they are interleaved ahead of the later-needed loads.
            wm_sb = wm_pool.tile([P, NKB1, DIM_H], bf16)
            nc.sync.dma_start(out=wm_sb, in_=wmd[:])
            for t in range(nbt):
                nc.sync.dma_start(out=xsT[:, t], in_=xtd[:, t])
            nc.sync.dma_start(out=bm_rep, in_=bcast_row(bmd[:]))
            load_wch(0)
            hx_all = hx_pool.tile([P, nbt, DIM_H], bf16)
            nc.sync.dma_start(
                out=hx_all, in_=hxd[:].rearrange("(t p) n -> p t n", p=P)
            )
            load_wch(1)
            for t in range(nbt):
                nc.sync.dma_start(out=hxT[:, t], in_=htd[:, t])
            nc.sync.dma_start(out=b_rep, in_=bcast_row(bd[:]))

            # ||hx||^2 per tile, early (ACT Square, accum)
            for t in range(nbt):
                dmp = dump_pool.tile([P, DIM_H], bf16, tag="dump", name=f"dhx{t}")
                nc.scalar.activation(
                    dmp, hx_all[:, t], AF.Square, accum_out=sqh_a[:, t : t + 1]
                )

            for t in range(nbt):
                # mapped = x @ Wm + bm   (2 chunks of 512)
                map_sb = map_pool.tile([P, DIM_H], fp32, tag="map", name=f"map{t}")
                for ch in range(2):
                    cs = slice(ch * CHUNK, (ch + 1) * CHUNK)
                    pm = ps_m1.tile([P, CHUNK], fp32, tag="pm", name=f"pm{t}_{ch}")
                    for kb in range(NKB1):
                        nc.tensor.matmul(
                            pm,
                            xsT[:, t, kb, :],
                            wm_sb[:, kb, cs],
                            start=(kb == 0),
                            stop=(kb == NKB1 - 1),
                        )
                    nc.vector.tensor_tensor(map_sb[:, cs], pm, bm_rep[:, cs], OP.add)
                # dot(mapped, hx) via STT with accumulate
                dmp = dump_pool.tile([P, DIM_H], bf16, tag="dump", name=f"ddot{t}")
                nc.vector.scalar_tensor_tensor(
                    out=dmp,
                    in0=map_sb,
                    scalar=1.0,
                    in1=hx_all[:, t],
                    op0=OP.mult,
                    op1=OP.mult,
                    accum_out=dot_a[:, t : t + 1],
                )
                # ||mapped||^2
                dmp2 = dump_pool.tile([P, DIM_H], bf16, tag="dump", name=f"dsq{t}")
                nc.scalar.activation(
                    dmp2, map_sb, AF.Square, accum_out=sqm_a[:, t : t + 1]
                )

                if t % 4 == 3:
                    # batched cosine finish for tiles t-3..t
                    g0 = t - 3
                    bs = slice(g0, t + 1)
                    m1 = sm_pool.tile([P, 4], fp32, tag="c_m1", name=f"m1_{t}")
                    nc.vector.tensor_scalar_max(m1, sqm_a[:, bs], COS_EPS2)
                    m2 = sm_pool.tile([P, 4], fp32, tag="c_m2", name=f"m2_{t}")
                    nc.vector.tensor_scalar_max(m2, sqh_a[:, bs], COS_EPS2)
                    den = sm_pool.tile([P, 4], fp32, tag="c_den", name=f"den_{t}")
                    nc.vector.tensor_tensor(den, m1, m2, OP.mult)
                    rinv = newton_rsqrt(den, 4, f"cos{t}")
                    cosv = sm_pool.tile([P, 4], fp32, tag="c_cos", name=f"cos_{t}")
                    nc.vector.tensor_tensor(cosv, dot_a[:, bs], rinv, OP.mult)
                    att = sm_pool.tile([P, 4], fp32, tag="c_att", name=f"att_{t}")
                    nc.scalar.activation(att, cosv, AF.Sigmoid)
                    nc.vector.tensor_scalar_add(s_a[:, bs], att, 1.0)

                    # fold s into xT for this tile group
                    for tt in range(g0, t + 1):
                        psT = ps_rep.tile([1, P], fp32, tag="psT", name=f"psT{tt}")
                        nc.tensor.transpose(psT, s_a[:, tt : tt + 1], ident)
                        nc.scalar.copy(zrow[0:1, :], psT)
                        psr = ps_rep.tile([P, P], fp32, tag="psr", name=f"psr{tt}")
                        nc.tensor.matmul(psr, ones128, zrow, start=True, stop=True)
                        srep = sm_pool.tile([P, P], bf16, tag="srep", name=f"sr{tt}")
                        nc.scalar.copy(srep, psr)
                        srep_brd = bass.AP(
                            tensor=srep.tensor,
                            offset=srep.offset,
                            ap=[list(srep.ap[0]), [0, NKB1], list(srep.ap[1])],
                        )
                        nc.vector.tensor_tensor(
                            xsT[:, tt], xsT[:, tt], srep_brd, OP.mult
                        )

        # ---------------- phase 2: gates ----------------
        # two alternating z pools so a wave's combines never wait on the
        # previous wave's apply passes to release slots
        z_pools = [
            ctx.enter_context(tc.tile_pool(name="z0", bufs=nbt)),
            ctx.enter_context(tc.tile_pool(name="z1", bufs=nbt)),
        ]
        gb_pool = ctx.enter_context(tc.tile_pool(name="gb", bufs=2))
        wk_pool = ctx.enter_context(tc.tile_pool(name="wk", bufs=3))
        ga_pool = ctx.enter_context(tc.tile_pool(name="ga", bufs=2))
        cx_pool = ctx.enter_context(tc.tile_pool(name="cxin", bufs=2))
        hxn_pool = ctx.enter_context(tc.tile_pool(name="hxn", bufs=2))
        ps_g = ctx.enter_context(tc.tile_pool(name="psg", bufs=6, space="PSUM"))

        def wave_tiles(name):
            return sm_pool.tile([P, nbt], fp32, tag=f"wv_{name}", name=name)

        wave_cids = {0: (0, 1), 2: (4, 5), 1: (2, 3), 3: (6, 7)}
        gate_order = (
            (0, AF.Sigmoid, "i"),
            (2, AF.Tanh, "g"),
            (1, AF.Sigmoid, "f"),
            (3, AF.Sigmoid, "o"),
        )

        for wi, (gi, func, role) in enumerate(gate_order):
            cids = wave_cids[gi]
            for cid in cids:
                load_wch(cid)
            # prefetch next wave's first chunk (3rd buffer)
            if wi + 1 < 4:
                load_wch(wave_cids[gate_order[wi + 1][0]][0])

            grep = gb_pool.tile([P, DIM_H], bf16, tag="grep", name=f"grep{gi}")
            nc.sync.dma_start(out=grep, in_=bcast_row(gd[gi, :]))
            brep = gb_pool.tile([P, DIM_H], bf16, tag="brep", name=f"brep{gi}")
            nc.sync.dma_start(out=brep, in_=bcast_row(btd[gi, :]))

            zs = [None] * nbt
            zsum = wave_tiles(f"zsum{gi}")  # per-chunk row sums
            zsum2 = wave_tiles(f"zsum2{gi}")
            zss = wave_tiles(f"zss{gi}")  # row sums of squares
            mean = wave_tiles(f"mean{gi}")
            msq = wave_tiles(f"msq{gi}")
            vare = wave_tiles(f"vare{gi}")

            for half in range(2):
                tl = range(half * (nbt // 2), (half + 1) * (nbt // 2))
                hs = slice(half * (nbt // 2), (half + 1) * (nbt // 2))
                for c, cid in enumerate(cids):
                    col0 = cid * CHUNK
                    for t in tl:
                        ps = ps_g.tile(
                            [P, CHUNK], fp32, tag="pg", name=f"pg{cid}_{t}"
                        )
                        for kb in range(NKB2):
                            lhsT = (
                                xsT[:, t, kb, :]
                                if kb < NKB1
                                else hxT[:, t, kb - NKB1, :]
                            )
                            nc.tensor.matmul(
                                ps,
                                lhsT,
                                wch[cid][:, kb, :],
                                start=(kb == 0),
                                stop=(kb == NKB2 - 1),
                            )
                        if c == 0:
                            zs[t] = z_pools[wi % 2].tile(
                                [P, DIM_H], bf16, tag="z", name=f"z{gi}_{t}"
                            )
                        # combine: z = ps + b (accumulate row-sum for the mean)
                        acc = zsum if c == 0 else zsum2
                        nc.vector.scalar_tensor_tensor(
                            out=zs[t][:, c * CHUNK : (c + 1) * CHUNK],
                            in0=ps,
                            scalar=0.0,
                            in1=b_rep[:, col0 : col0 + CHUNK],
                            op0=OP.add,
                            op1=OP.add,
                            accum_out=acc[:, t : t + 1],
                        )

                # E[z^2] per tile via ACT Square accum
                for t in tl:
                    dmp = dump_pool.tile(
                        [P, DIM_H], bf16, tag="dump", name=f"dz{gi}{t}"
                    )
                    nc.scalar.activation(
                        dmp, zs[t], AF.Square, accum_out=zss[:, t : t + 1]
                    )

                # batched LN stats for this half on [P,4] slices
                nc.vector.tensor_tensor(mean[:, hs], zsum[:, hs], zsum2[:, hs], OP.add)
                nc.vector.tensor_scalar_mul(mean[:, hs], mean[:, hs], 1.0 / DIM_H)
                nc.vector.tensor_tensor(msq[:, hs], mean[:, hs], mean[:, hs], OP.mult)
                nc.vector.scalar_tensor_tensor(
                    out=vare[:, hs],
                    in0=zss[:, hs],
                    scalar=1.0 / DIM_H,
                    in1=msq[:, hs],
                    op0=OP.mult,
                    op1=OP.subtract,
                )
                nc.vector.tensor_scalar_add(vare[:, hs], vare[:, hs], LN_EPS)
                rstd = newton_rsqrt(vare[:, hs], nbt // 2, f"ln{gi}h{half}")

                for j, t in enumerate(tl):
                    if role == "i":
                        ga = iact[t]
                    else:
                        ga = ga_pool.tile(
                            [P, DIM_H], fp32, tag="ga", name=f"ga{gi}_{t}"
                        )
                    if role == "o":
                        # tail gate: keep everything on DVE (Pool latency
                        # would sit exposed after the last matmuls)
                        wk = wk_pool.tile(
                            [P, DIM_H], bf16, tag="wk", name=f"wk{gi}_{t}"
                        )
                        nc.vector.scalar_tensor_tensor(
                            out=wk,
                            in0=zs[t],
                            scalar=mean[:, t : t + 1],
                            in1=grep,
                            op0=OP.subtract,
                            op1=OP.mult,
                        )
                        nc.vector.scalar_tensor_tensor(
                            out=ga,
                            in0=wk,
                            scalar=rstd[:, j : j + 1],
                            in1=brep,
                            op0=OP.mult,
                            op1=OP.add,
                        )
                    else:
                        # u = (z - mean) * rstd on DVE, affine on Pool
                        wk = wk_pool.tile(
                            [P, DIM_H], bf16, tag="wk", name=f"wk{gi}_{t}"
                        )
                        nc.vector.tensor_scalar(
                            wk,
                            zs[t],
                            mean[:, t : t + 1],
                            rstd[:, j : j + 1],
                            OP.subtract,
                            OP.mult,
                        )
                        nc.gpsimd.tensor_tensor(ga, wk, grep, OP.mult)
                        nc.gpsimd.tensor_tensor(ga, ga, brep, OP.add)
                    nc.scalar.activation(ga, ga, func)

                    if role == "g":
                        nc.vector.tensor_tensor(iact[t], iact[t], ga, OP.mult)
                    elif role == "f":
                        cx_t = cx_pool.tile(
                            [P, DIM_H], fp32, tag="cx", name=f"cx{t}"
                        )
                        nc.sync.dma_start(
                            out=cx_t, in_=cxd[t * P : (t + 1) * P, :]
                        )
                        nc.gpsimd.tensor_tensor(cx_t, ga, cx_t, OP.mult)
                        nc.gpsimd.tensor_tensor(iact[t], iact[t], cx_t, OP.add)
                        nc.scalar.dma_start(
                            out=cxo[t * P : (t + 1) * P, :], in_=iact[t]
                        )
                    elif role == "o":
                        # hx_new = o * tanh(cx_new)
                        tnh = hxn_pool.tile(
                            [P, DIM_H], fp32, tag="tnh", name=f"tnh{t}"
                        )
                        nc.scalar.activation(tnh, iact[t], AF.Tanh)
                        hxn = tnh
                        nc.gpsimd.tensor_tensor(hxn, ga, tnh, OP.mult)
                        # second cosine gate
                        dot2 = sm_pool.tile([P, 1], fp32, tag="o_dot", name=f"d2_{t}")
                        dmp = dump_pool.tile(
                            [P, DIM_H], bf16, tag="dump", name=f"do{t}"
                        )
                        nc.vector.scalar_tensor_tensor(
                            out=dmp,
                            in0=hxn,
                            scalar=1.0,
                            in1=iact[t],
                            op0=OP.mult,
                            op1=OP.mult,
                            accum_out=dot2,
                        )
                        sq1 = sm_pool.tile([P, 1], fp32, tag="o_sq1", name=f"s1_{t}")
                        dmpa = dump_pool.tile(
                            [P, DIM_H], bf16, tag="dump", name=f"da{t}"
                        )
                        nc.scalar.activation(dmpa, hxn, AF.Square, accum_out=sq1)
                        sq2 = sm_pool.tile([P, 1], fp32, tag="o_sq2", name=f"s2_{t}")
                        dmpb = dump_pool.tile(
                            [P, DIM_H], bf16, tag="dump", name=f"db{t}"
                        )
                        nc.scalar.activation(dmpb, iact[t], AF.Square, accum_out=sq2)
                        ma = sm_pool.tile([P, 1], fp32, tag="o_ma", name=f"ma_{t}")
                        nc.vector.tensor_scalar_max(ma, sq1, COS_EPS2)
                        mb = sm_pool.tile([P, 1], fp32, tag="o_mb", name=f"mb_{t}")
                        nc.vector.tensor_scalar_max(mb, sq2, COS_EPS2)
                        dn2 = sm_pool.tile([P, 1], fp32, tag="o_dn", name=f"dn_{t}")
                        nc.vector.tensor_tensor(dn2, ma, mb, OP.mult)
                        rr2 = newton_rsqrt(dn2, 1, f"o{t}")
                        cos2 = sm_pool.tile([P, 1], fp32, tag="o_cs", name=f"cs_{t}")
                        nc.vector.tensor_tensor(cos2, dot2, rr2, OP.mult)
                        co = sm_pool.tile([P, 1], fp32, tag="o_co", name=f"co_{t}")
                        # sigmoid((cos+1)/2) = sigmoid(0.5*cos + 0.5)
                        nc.scalar.activation(
                            co, cos2, AF.Sigmoid, bias=halfc, scale=0.5
                        )
                        cop = sm_pool.tile([P, 1], fp32, tag="o_cp", name=f"cp_{t}")
                        nc.vector.tensor_scalar_add(cop, co, 1.0)
                        # final scale on Pool: gpsimd can't take an AP scalar,
                        # so replicate (1+co) along the free dim via 0-step AP
                        cop_brd = bass.AP(
                            tensor=cop.tensor,
                            offset=cop.offset,
                            ap=[list(cop.ap[0]), [0, DIM_H]],
                        )
                        nc.gpsimd.tensor_tensor(hxn, hxn, cop_brd, OP.mult)
                        nc.scalar.dma_start(
                            out=hxo[t * P : (t + 1) * P, :], in_=hxn
                        )
    _split_excess_waits(nc)
    return nc


def _split_excess_waits(nc):
    """Walrus ISA structs have limited sync-wait slots (Matmult/LDW: 1,
    DMA: 2, several DVE/ACT structs: 1-2). The Tile scheduler can emit more.
    Move excess waits onto standalone EventSemaphore instructions injected
    just before the offender on the same engine."""
    import concourse.mybir as mybir

    caps = {}
    skip = {"EventSemaphore", "RegisterMove", "UnconditionalBranch"}
    n_split = 0
    for fn in nc.m.functions:
        for blk in fn.blocks:
            out = []
            changed = False
            for ins in blk.instructions:
                si = ins.sync_info
                opname = type(ins).__name__.replace("Inst", "", 1)
                if (
                    si is not None
                    and si.on_wait
                    and opname not in skip
                    and len(si.on_wait) > caps.get(opname, 1)
                ):
                    cap = caps.get(opname, 1)
                    waits = list(si.on_wait)
                    excess, keep = waits[:-cap], waits[-cap:]
                    for k, w in enumerate(excess):
                        ev = mybir.InstEventSemaphore(
                            name=f"{ins.name}-wsp{k}",
                            ins=[],
                            outs=[],
                            sync_info=mybir.SyncInfo(on_wait=[w], on_update=[]),
                        )
                        ev.engine = ins.engine
                        out.append(ev)
                        n_split += 1
                    ins.sync_info = mybir.SyncInfo(
                        on_wait=keep, on_update=list(si.on_update)
                    )
                    changed = True
                out.append(ins)
            if changed:
                blk.instructions = out
    return n_split


def _get_nc():
    if "nc" not in _cache:
        _cache["nc"] = build_nc()
    return _cache["nc"]


def _pack_inputs(x, hx, cx, W, b, Wm, bm, gammas, betas):
    """Host-side packing: bf16 casts + transposed tile layouts."""
    import ml_dtypes

    bf = ml_dtypes.bfloat16
    xb = np.asarray(x, np.float32).astype(bf)
    hxb = np.asarray(hx, np.float32).astype(bf)
    cxf = np.ascontiguousarray(np.asarray(cx, np.float32))
    # W[kb*128+p, c*512+n] -> wp4[p, c, kb, n]
    wp4 = np.ascontiguousarray(
        np.asarray(W, np.float32)
        .astype(bf)
        .reshape(NKB2, P, NCHUNK, CHUNK)
        .transpose(1, 2, 0, 3)
    )
    # Wm[kb*128+p, n] -> wmp[p, kb, n]
    wmp = np.ascontiguousarray(
        np.asarray(Wm, np.float32).astype(bf).reshape(NKB1, P, DIM_H).transpose(1, 0, 2)
    )
    shared = {
        "wp4": wp4,
        "wmp": wmp,
        "bb": np.ascontiguousarray(np.asarray(b, np.float32).astype(bf)),
        "bmb": np.ascontiguousarray(np.asarray(bm, np.float32).astype(bf)),
        "gmb": np.ascontiguousarray(np.asarray(gammas, np.float32).astype(bf)),
        "btb": np.ascontiguousarray(np.asarray(betas, np.float32).astype(bf)),
    }
    in_maps = []
    for i in range(NCORES):
        sl = slice(i * BL, (i + 1) * BL)
        xc, hc = xb[sl], hxb[sl]
        # x[t*128+r, kb*128+p] -> xt4[p, t, kb, r]
        xt4 = np.ascontiguousarray(
            xc.reshape(NBT, P, NKB1, P).transpose(3, 0, 2, 1)
        )
        ht4 = np.ascontiguousarray(
            hc.reshape(NBT, P, NKB1, P).transpose(3, 0, 2, 1)
        )
        in_maps.append(
            {
                "xt4": xt4,
                "ht4": ht4,
                "hxr": np.ascontiguousarray(hc),
                "cxr": cxf[sl],
                **shared,
            }
        )
    return in_maps


def kernel(x, hx, cx, W, b, Wm, bm, gammas, betas):
    from concourse.bass_utils import run_bass_kernel_spmd

    nc = _get_nc()
    in_maps = _pack_inputs(x, hx, cx, W, b, Wm, bm, gammas, betas)
    res = run_bass_kernel_spmd(nc, in_maps, list(range(NCORES)))
    hx_mod = np.concatenate([r["hx_out"] for r in res.results], axis=0)
    cx_new = np.concatenate([r["cx_out"] for r in res.results], axis=0)
    return (hx_mod, cx_new)


# revision 28
# speedup vs baseline: 1.0617x; 1.0617x over previous
"""Trainium2 Bass kernel for the cosine-gated LSTM cell (CGLSTMCellv1).

Full inputs in, full outputs out. Data-parallel: batch sharded across 8
NeuronCores, weights replicated, no cross-core comms.

Key performance choices vs the fp32 baseline:
  - All matmuls in bf16 (fp32 matmul runs at 4 cycles/row on TRN2 PE; bf16
    at 1). Operands are cast and laid out host-side.
  - x^T / hx^T are packed on the host into the exact SBUF tile layout
    [p, t, kb, r], eliminating all 128 PE transposes + PSUM round-trips.
  - W is host-packed per column-chunk [p, c, kb, n] so each chunk load is
    one contiguous 16KB-per-partition DMA; chunks are triple-buffered.
  - DMAs are issued in first-use order (Wm + x^T tiles first) so the first
    matmul starts ~10us in, not after all input DMAs.
  - LayerNorm rstd and the two cosine denominators use a DVE-only
    Newton-iteration rsqrt (bit hack + 2 iterations) instead of ACT Sqrt,
    so the scalar engine needs exactly one activation-table set
    (sigmoid_and_others: sigmoid/tanh/square/copy) -> one table load total.
  - Gate bias b is added during the PSUM->SBUF combine pass (free).
  - LN mean comes free from the combine pass accum; E[z^2] from an ACT
    Square accum pass; stats are batched per half-wave (4 tiles) on [P,4]
    tiles to shorten the stats barrier.
  - The o-gate's normalize/affine work runs on DVE (not Pool) because it
    sits in the post-matmul tail where Pool's 2.3us/op latency would be
    exposed; the earlier gates' affine runs on Pool to offload DVE while
    the PE still has matmul work to overlap.

Math per core (rows = local batch slice):
  mapped = x @ Wm + bm
  attn   = sigmoid(cos_sim(mapped, hx));  s = 1 + attn
  gates  = concat(s*x, hx) @ W + b   (s folded into x^T)
  i,f,g,o = LN-gates -> sigmoid/tanh
  cx_new = f*cx + i*g ; hx_new = o*tanh(cx_new)
  hx_mod = hx_new * (1 + sigmoid((cos_sim(hx_new,cx_new)+1)/2))
"""

import numpy as np

B_FULL, DIM_I, DIM_H = 8192, 1024, 1024
NCORES = 8
BL = B_FULL // NCORES  # 1024 rows per core
P = 128
H4 = 4 * DIM_H
NBT = BL // P               # 8 row tiles of 128
NKB1 = DIM_I // P           # 8  k-blocks for the x part
NKB2 = (DIM_I + DIM_H) // P  # 16 k-blocks for mm2
CHUNK = 512                 # W column chunk (= one PSUM bank of fp32)
NCHUNK = H4 // CHUNK        # 8 chunks total, 2 per gate
LN_EPS = 1e-5
COS_EPS2 = 1e-12
MAGIC = 0x5F3759DF

_cache = {}


def build_nc(nbt=NBT):
    from contextlib import ExitStack

    import concourse.bass as bass
    import concourse.mybir as mybir
    import concourse.tile as tile

    fp32 = mybir.dt.float32
    bf16 = mybir.dt.bfloat16
    i32 = mybir.dt.int32
    AF = mybir.ActivationFunctionType
    OP = mybir.AluOpType
    bl = nbt * P

    nc = bass.Bass()
    # host-packed inputs (see kernel() for the packing)
    xtd = nc.dram_tensor("xt4", [P, nbt, NKB1, P], bf16, kind="ExternalInput")
    htd = nc.dram_tensor("ht4", [P, nbt, NKB1, P], bf16, kind="ExternalInput")
    hxd = nc.dram_tensor("hxr", [bl, DIM_H], bf16, kind="ExternalInput")
    cxd = nc.dram_tensor("cxr", [bl, DIM_H], fp32, kind="ExternalInput")
    wpd = nc.dram_tensor("wp4", [P, NCHUNK, NKB2, CHUNK], bf16, kind="ExternalInput")
    wmd = nc.dram_tensor("wmp", [P, NKB1, DIM_H], bf16, kind="ExternalInput")
    bd = nc.dram_tensor("bb", [H4], bf16, kind="ExternalInput")
    bmd = nc.dram_tensor("bmb", [DIM_H], bf16, kind="ExternalInput")
    gd = nc.dram_tensor("gmb", [4, DIM_H], bf16, kind="ExternalInput")
    btd = nc.dram_tensor("btb", [4, DIM_H], bf16, kind="ExternalInput")
    hxo = nc.dram_tensor("hx_out", [bl, DIM_H], fp32, kind="ExternalOutput")
    cxo = nc.dram_tensor("cx_out", [bl, DIM_H], fp32, kind="ExternalOutput")

    def bcast_row(src_ap):
        # view an [N]-shaped AP as [P, N] with 0-step partition broadcast
        return bass.AP(
            tensor=src_ap.tensor, offset=src_ap.offset, ap=[[0, P]] + list(src_ap.ap)
        )

    with tile.TileContext(nc) as tc, ExitStack() as ctx:
        singles = ctx.enter_context(tc.tile_pool(name="singles", bufs=1))
        sm_pool = ctx.enter_context(tc.tile_pool(name="smalls", bufs=4))

        from concourse.masks import make_identity

        ident = singles.tile([P, P], fp32)
        ones128 = singles.tile([P, P], fp32)
        zrow = singles.tile([P, P], fp32)
        magic_t = singles.tile([P, 8], i32)
        halfc = singles.tile([P, 1], fp32)
        junk = singles.tile([P, 1], fp32)

        def emit_consts():
            make_identity(nc, ident)
            nc.vector.memset(ones128, 1.0)
            nc.vector.memset(zrow, 0.0)
            nc.vector.memset(magic_t, MAGIC)
            nc.vector.memset(halfc, 0.5)
            nc.vector.memset(junk, 0.5)
            # prime the ACT table with the one set the whole kernel uses
            nc.scalar.activation(junk, junk, AF.Sigmoid)

        def newton_rsqrt(x_f32, n, name):
            """y ~= 1/sqrt(x) on DVE only: bit hack + 2 Newton steps."""
            xi = x_f32.bitcast(i32)
            sh = sm_pool.tile([P, n], i32, tag="nr_sh", name=f"sh_{name}")
            nc.vector.tensor_scalar(sh, xi, 1, None, op0=OP.logical_shift_right)
            yi = sm_pool.tile([P, n], i32, tag="nr_yi", name=f"yi_{name}")
            nc.vector.tensor_tensor(yi, magic_t[:, 0:n], sh, OP.subtract)
            y = yi.bitcast(fp32)
            for it in range(2):
                a = sm_pool.tile([P, n], fp32, tag="nr_a", name=f"a{it}_{name}")
                nc.vector.tensor_tensor(a, y, y, OP.mult)
                nc.vector.tensor_tensor(a, a, x_f32, OP.mult)
                nc.vector.tensor_scalar(a, a, -0.5, 1.5, OP.mult, OP.add)
                yn = sm_pool.tile([P, n], fp32, tag="nr_y", name=f"y{it}_{name}")
                nc.vector.tensor_tensor(yn, y, a, OP.mult)
                y = yn
            return y

        # persistent transposed activations (xsT scaled in place after mm1)
        xsT = singles.tile([P, nbt, NKB1, P], bf16)
        hxT = singles.tile([P, nbt, NKB1, P], bf16)
        bm_rep = singles.tile([P, DIM_H], bf16)
        b_rep = singles.tile([P, H4], bf16)

        # cos-gate scratch, batched across tiles as [P, nbt] columns
        dot_a = sm_pool.tile([P, nbt], fp32, tag="dot_a")
        sqm_a = sm_pool.tile([P, nbt], fp32, tag="sqm_a")
        sqh_a = sm_pool.tile([P, nbt], fp32, tag="sqh_a")
        s_a = sm_pool.tile([P, nbt], fp32, tag="s_a")

        iact_pool = ctx.enter_context(tc.tile_pool(name="iact", bufs=nbt))
        iact = [
            iact_pool.tile([P, DIM_H], fp32, tag="iact", name=f"iact{t}")
            for t in range(nbt)
        ]
        w_pool = ctx.enter_context(tc.tile_pool(name="wch", bufs=2))
        dump_pool = ctx.enter_context(tc.tile_pool(name="dump", bufs=4))
        ps_rep = ctx.enter_context(tc.tile_pool(name="psrep", bufs=2, space="PSUM"))
        wch = {}

        def load_wch(cid):
            if cid in wch or cid >= NCHUNK:
                return
            wch[cid] = w_pool.tile(
                [P, NKB2, CHUNK], bf16, tag="wch", name=f"wch{cid}"
            )
            nc.sync.dma_start(out=wch[cid], in_=wpd[:, cid])

        # s-fold machinery: replicate s across partitions via PE, then
        # scale x^T in place. Emitted at points where the PE would
        # otherwise be busy so the in-order PE stream never stalls on it.
        def fold_s(tiles):
            for tt in tiles:
                psT = ps_rep.tile([1, P], fp32, tag="rep", name=f"psT{tt}")
                nc.tensor.transpose(psT, s_a[:, tt : tt + 1], ident)
                nc.scalar.copy(zrow[0:1, :], psT)
                psr = ps_rep.tile([P, P], fp32, tag="rep", name=f"psr{tt}")
                nc.tensor.matmul(psr, ones128, zrow, start=True, stop=True)
                srep = sm_pool.tile([P, P], bf16, tag="srep", name=f"sr{tt}")
                nc.scalar.copy(srep, psr)
                srep_brd = bass.AP(
                    tensor=srep.tensor,
                    offset=srep.offset,
                    ap=[list(srep.ap[0]), [0, NKB1], list(srep.ap[1])],
                )
                nc.vector.tensor_tensor(xsT[:, tt], xsT[:, tt], srep_brd, OP.mult)

        # ---------------- phase 1: mm1 + cosine gate ----------------
        with ExitStack() as p1:
            wm_pool = p1.enter_context(tc.tile_pool(name="wm", bufs=1))
            hx_pool = p1.enter_context(tc.tile_pool(name="hxr", bufs=1))
            map_pool = p1.enter_context(tc.tile_pool(name="map", bufs=3))
            ps_m1 = p1.enter_context(tc.tile_pool(name="psm1", bufs=2, space="PSUM"))

            # DMAs first, in first-use order, before any const setup: Wm and
            # x^T tiles gate the first matmuls; W chunks 0/1 and hx^T must
            # land before the first wave (~35us).
            wm_sb = wm_pool.tile([P, NKB1, DIM_H], bf16)
            nc.sync.dma_start(out=wm_sb, in_=wmd[:])
            for t in range(nbt):
                nc.sync.dma_start(out=xsT[:, t], in_=xtd[:, t])
            nc.sync.dma_start(out=bm_rep, in_=bcast_row(bmd[:]))
            load_wch(0)
            hx_all = hx_pool.tile([P, nbt, DIM_H], bf16)
            nc.sync.dma_start(
                out=hx_all, in_=hxd[:].rearrange("(t p) n -> p t n", p=P)
            )
            for t in range(nbt):
                nc.sync.dma_start(out=hxT[:, t], in_=htd[:, t])
            load_wch(1)
            nc.sync.dma_start(out=b_rep, in_=bcast_row(bd[:]))
            emit_consts()

            # ||hx||^2 per tile, early (ACT Square, accum)
            for t in range(nbt):
                dmp = dump_pool.tile([P, DIM_H], bf16, tag="dump", name=f"dhx{t}")
                nc.scalar.activation(
                    dmp, hx_all[:, t], AF.Square, accum_out=sqh_a[:, t : t + 1]
                )

            for t in range(nbt):
                # mapped = x @ Wm + bm   (2 chunks of 512)
                map_sb = map_pool.tile([P, DIM_H], fp32, tag="map", name=f"map{t}")
                for ch in range(2):
                    cs = slice(ch * CHUNK, (ch + 1) * CHUNK)
                    pm = ps_m1.tile([P, CHUNK], fp32, tag="pm", name=f"pm{t}_{ch}")
                    for kb in range(NKB1):
                        nc.tensor.matmul(
                            pm,
                            xsT[:, t, kb, :],
                            wm_sb[:, kb, cs],
                            start=(kb == 0),
                            stop=(kb == NKB1 - 1),
                        )
                    nc.vector.tensor_tensor(map_sb[:, cs], pm, bm_rep[:, cs], OP.add)
                # dot(mapped, hx) via STT with accumulate
                dmp = dump_pool.tile([P, DIM_H], bf16, tag="dump", name=f"ddot{t}")
                nc.vector.scalar_tensor_tensor(
                    out=dmp,
                    in0=map_sb,
                    scalar=1.0,
                    in1=hx_all[:, t],
                    op0=OP.mult,
                    op1=OP.mult,
                    accum_out=dot_a[:, t : t + 1],
                )
                # ||mapped||^2
                dmp2 = dump_pool.tile([P, DIM_H], bf16, tag="dump", name=f"dsq{t}")
                nc.scalar.activation(
                    dmp2, map_sb, AF.Square, accum_out=sqm_a[:, t : t + 1]
                )

                if t % 4 == 3:
                    # batched cosine finish for tiles t-3..t
                    g0 = t - 3
                    bs = slice(g0, t + 1)
                    m1 = sm_pool.tile([P, 4], fp32, tag="c_m1", name=f"m1_{t}")
                    nc.vector.tensor_scalar_max(m1, sqm_a[:, bs], COS_EPS2)
                    m2 = sm_pool.tile([P, 4], fp32, tag="c_m2", name=f"m2_{t}")
                    nc.vector.tensor_scalar_max(m2, sqh_a[:, bs], COS_EPS2)
                    den = sm_pool.tile([P, 4], fp32, tag="c_den", name=f"den_{t}")
                    nc.vector.tensor_tensor(den, m1, m2, OP.mult)
                    rinv = newton_rsqrt(den, 4, f"cos{t}")
                    cosv = sm_pool.tile([P, 4], fp32, tag="c_cos", name=f"cos_{t}")
                    nc.vector.tensor_tensor(cosv, dot_a[:, bs], rinv, OP.mult)
                    att = sm_pool.tile([P, 4], fp32, tag="c_att", name=f"att_{t}")
                    nc.scalar.activation(att, cosv, AF.Sigmoid)
                    nc.vector.tensor_scalar_add(s_a[:, bs], att, 1.0)
                if t == 5:
                    # batch-A fold lands while mm1 t6/t7 keeps the PE busy
                    fold_s(range(0, 4))

        # ---------------- phase 2: gates ----------------
        # two alternating z pools so a wave's combines never wait on the
        # previous wave's apply passes to release slots
        z_pools = [
            ctx.enter_context(tc.tile_pool(name="z0", bufs=nbt)),
            ctx.enter_context(tc.tile_pool(name="z1", bufs=nbt)),
        ]
        gb_pool = ctx.enter_context(tc.tile_pool(name="gb", bufs=2))
        wk_pool = ctx.enter_context(tc.tile_pool(name="wk", bufs=3))
        ga_pool = ctx.enter_context(tc.tile_pool(name="ga", bufs=2))
        cx_pool = ctx.enter_context(tc.tile_pool(name="cxin", bufs=2))
        hxn_pool = ctx.enter_context(tc.tile_pool(name="hxn", bufs=4))
        ps_g = ctx.enter_context(tc.tile_pool(name="psg", bufs=6, space="PSUM"))

        def wave_tiles(name):
            return sm_pool.tile([P, nbt], fp32, tag=f"wv_{name}", name=name)

        wave_cids = {0: (0, 1), 2: (4, 5), 1: (2, 3), 3: (6, 7)}
        gate_order = (
            (0, AF.Sigmoid, "i"),
            (2, AF.Tanh, "g"),
            (1, AF.Sigmoid, "f"),
            (3, AF.Sigmoid, "o"),
        )

        for wi, (gi, func, role) in enumerate(gate_order):
            cids = wave_cids[gi]
            for cid in cids:
                load_wch(cid)
            # prefetch next wave's first chunk (3rd buffer)
            if wi + 1 < 4:
                load_wch(wave_cids[gate_order[wi + 1][0]][0])

            grep = gb_pool.tile([P, DIM_H], bf16, tag="grep", name=f"grep{gi}")
            nc.sync.dma_start(out=grep, in_=bcast_row(gd[gi, :]))
            brep = gb_pool.tile([P, DIM_H], bf16, tag="brep", name=f"brep{gi}")
            nc.sync.dma_start(out=brep, in_=bcast_row(btd[gi, :]))

            zs = [None] * nbt
            zsum = wave_tiles(f"zsum{gi}")  # per-chunk row sums
            zsum2 = wave_tiles(f"zsum2{gi}")
            zss = wave_tiles(f"zss{gi}")  # row sums of squares
            mean = wave_tiles(f"mean{gi}")
            msq = wave_tiles(f"msq{gi}")
            vare = wave_tiles(f"vare{gi}")

            if role == "o":
                d2w = wave_tiles("o_d2")
                s1w = wave_tiles("o_s1")
                s2w = wave_tiles("o_s2")
                hxns = {}

            for half in range(2):
                tl = range(half * (nbt // 2), (half + 1) * (nbt // 2))
                hs = slice(half * (nbt // 2), (half + 1) * (nbt // 2))
                if wi == 0 and half == 1:
                    # batch-B s-fold: the PE is busy with wave-0 half-0
                    # matmuls while the batch-B cosine chain completes
                    fold_s(range(4, 8))
                for c, cid in enumerate(cids):
                    col0 = cid * CHUNK
                    for t in tl:
                        ps = ps_g.tile(
                            [P, CHUNK], fp32, tag="pg", name=f"pg{cid}_{t}"
                        )
                        for kb in range(NKB2):
                            lhsT = (
                                xsT[:, t, kb, :]
                                if kb < NKB1
                                else hxT[:, t, kb - NKB1, :]
                            )
                            nc.tensor.matmul(
                                ps,
                                lhsT,
                                wch[cid][:, kb, :],
                                start=(kb == 0),
                                stop=(kb == NKB2 - 1),
                            )
                        if c == 0:
                            zs[t] = z_pools[wi % 2].tile(
                                [P, DIM_H], bf16, tag="z", name=f"z{gi}_{t}"
                            )
                        # combine: z = ps + b (accumulate row-sum for the mean)
                        acc = zsum if c == 0 else zsum2
                        nc.vector.scalar_tensor_tensor(
                            out=zs[t][:, c * CHUNK : (c + 1) * CHUNK],
                            in0=ps,
                            scalar=0.0,
                            in1=b_rep[:, col0 : col0 + CHUNK],
                            op0=OP.add,
                            op1=OP.add,
                            accum_out=acc[:, t : t + 1],
                        )

                # E[z^2] per tile via ACT Square accum
                for t in tl:
                    dmp = dump_pool.tile(
                        [P, DIM_H], bf16, tag="dump", name=f"dz{gi}{t}"
                    )
                    nc.scalar.activation(
                        dmp, zs[t], AF.Square, accum_out=zss[:, t : t + 1]
                    )

                # batched LN stats for this half on [P,4] slices
                nc.vector.tensor_tensor(mean[:, hs], zsum[:, hs], zsum2[:, hs], OP.add)
                nc.vector.tensor_scalar_mul(mean[:, hs], mean[:, hs], 1.0 / DIM_H)
                nc.vector.tensor_tensor(msq[:, hs], mean[:, hs], mean[:, hs], OP.mult)
                nc.vector.scalar_tensor_tensor(
                    out=vare[:, hs],
                    in0=zss[:, hs],
                    scalar=1.0 / DIM_H,
                    in1=msq[:, hs],
                    op0=OP.mult,
                    op1=OP.subtract,
                )
                nc.vector.tensor_scalar_add(vare[:, hs], vare[:, hs], LN_EPS)
                rstd = newton_rsqrt(vare[:, hs], nbt // 2, f"ln{gi}h{half}")

                for j, t in enumerate(tl):
                    if role == "i":
                        ga = iact[t]
                    else:
                        ga = ga_pool.tile(
                            [P, DIM_H], fp32, tag="ga", name=f"ga{gi}_{t}"
                        )
                    if role == "o":
                        # tail gate: keep everything on DVE (Pool latency
                        # would sit exposed after the last matmuls)
                        wk = wk_pool.tile(
                            [P, DIM_H], bf16, tag="wk", name=f"wk{gi}_{t}"
                        )
                        nc.vector.scalar_tensor_tensor(
                            out=wk,
                            in0=zs[t],
                            scalar=mean[:, t : t + 1],
                            in1=grep,
                            op0=OP.subtract,
                            op1=OP.mult,
                        )
                        nc.vector.scalar_tensor_tensor(
                            out=ga,
                            in0=wk,
                            scalar=rstd[:, j : j + 1],
                            in1=brep,
                            op0=OP.mult,
                            op1=OP.add,
                        )
                    else:
                        # u = (z - mean) * rstd on DVE, affine on Pool
                        wk = wk_pool.tile(
                            [P, DIM_H], bf16, tag="wk", name=f"wk{gi}_{t}"
                        )
                        nc.vector.tensor_scalar(
                            wk,
                            zs[t],
                            mean[:, t : t + 1],
                            rstd[:, j : j + 1],
                            OP.subtract,
                            OP.mult,
                        )
                        nc.gpsimd.tensor_tensor(ga, wk, grep, OP.mult)
                        nc.gpsimd.tensor_tensor(ga, ga, brep, OP.add)
                    nc.scalar.activation(ga, ga, func)

                    if role == "g":
                        nc.vector.tensor_tensor(iact[t], iact[t], ga, OP.mult)
                    elif role == "f":
                        cx_t = cx_pool.tile(
                            [P, DIM_H], fp32, tag="cx", name=f"cx{t}"
                        )
                        nc.sync.dma_start(
                            out=cx_t, in_=cxd[t * P : (t + 1) * P, :]
                        )
                        nc.gpsimd.tensor_tensor(cx_t, ga, cx_t, OP.mult)
                        nc.gpsimd.tensor_tensor(iact[t], iact[t], cx_t, OP.add)
                        nc.scalar.dma_start(
                            out=cxo[t * P : (t + 1) * P, :], in_=iact[t]
                        )
                    elif role == "o":
                        # hx_new = o * tanh(cx_new)
                        tnh = hxn_pool.tile(
                            [P, DIM_H], fp32, tag="tnh", name=f"tnh{t}"
                        )
                        nc.scalar.activation(tnh, iact[t], AF.Tanh)
                        hxn = tnh
                        nc.gpsimd.tensor_tensor(hxn, ga, tnh, OP.mult)
                        hxns[t] = hxn
                        # second cosine gate: accumulate dot/norms per tile
                        dmp = dump_pool.tile(
                            [P, DIM_H], bf16, tag="dump", name=f"do{t}"
                        )
                        nc.vector.scalar_tensor_tensor(
                            out=dmp,
                            in0=hxn,
                            scalar=1.0,
                            in1=iact[t],
                            op0=OP.mult,
                            op1=OP.mult,
                            accum_out=d2w[:, t : t + 1],
                        )
                        dmpa = dump_pool.tile(
                            [P, DIM_H], bf16, tag="dump", name=f"da{t}"
                        )
                        nc.scalar.activation(
                            dmpa, hxn, AF.Square, accum_out=s1w[:, t : t + 1]
                        )
                        dmpb = dump_pool.tile(
                            [P, DIM_H], bf16, tag="dump", name=f"db{t}"
                        )
                        nc.scalar.activation(
                            dmpb, iact[t], AF.Square, accum_out=s2w[:, t : t + 1]
                        )

                if role == "o":
                    # batched second-cosine finish for this half's 4 tiles
                    ma = sm_pool.tile([P, 4], fp32, tag="o_ma", name=f"ma_{half}")
                    nc.vector.tensor_scalar_max(ma, s1w[:, hs], COS_EPS2)
                    mb = sm_pool.tile([P, 4], fp32, tag="o_mb", name=f"mb_{half}")
                    nc.vector.tensor_scalar_max(mb, s2w[:, hs], COS_EPS2)
                    dn2 = sm_pool.tile([P, 4], fp32, tag="o_dn", name=f"dn_{half}")
                    nc.vector.tensor_tensor(dn2, ma, mb, OP.mult)
                    rr2 = newton_rsqrt(dn2, 4, f"o{half}")
                    cos2 = sm_pool.tile([P, 4], fp32, tag="o_cs", name=f"cs_{half}")
                    nc.vector.tensor_tensor(cos2, d2w[:, hs], rr2, OP.mult)
                    co = sm_pool.tile([P, 4], fp32, tag="o_co", name=f"co_{half}")
                    # sigmoid((cos+1)/2) = sigmoid(0.5*cos + 0.5)
                    nc.scalar.activation(co, cos2, AF.Sigmoid, bias=halfc, scale=0.5)
                    cop = sm_pool.tile([P, 4], fp32, tag="o_cp", name=f"cp_{half}")
                    nc.vector.tensor_scalar_add(cop, co, 1.0)
                    for j, t in enumerate(tl):
                        # final scale on Pool: gpsimd can't take an AP scalar,
                        # so replicate (1+co) along the free dim via 0-step AP
                        cop_col = cop[:, j : j + 1]
                        cop_brd = bass.AP(
                            tensor=cop_col.tensor,
                            offset=cop_col.offset,
                            ap=[list(cop_col.ap[0]), [0, DIM_H]],
                        )
                        nc.gpsimd.tensor_tensor(hxns[t], hxns[t], cop_brd, OP.mult)
                        nc.scalar.dma_start(
                            out=hxo[t * P : (t + 1) * P, :], in_=hxns[t]
                        )
    _split_excess_waits(nc)
    return nc


def _split_excess_waits(nc):
    """Walrus ISA structs have limited sync-wait slots (Matmult/LDW: 1,
    DMA: 2, several DVE/ACT structs: 1-2). The Tile scheduler can emit more.
    Move excess waits onto standalone EventSemaphore instructions injected
    just before the offender on the same engine."""
    import concourse.mybir as mybir

    caps = {}
    skip = {"EventSemaphore", "RegisterMove", "UnconditionalBranch"}
    n_split = 0
    for fn in nc.m.functions:
        for blk in fn.blocks:
            out = []
            changed = False
            for ins in blk.instructions:
                si = ins.sync_info
                opname = type(ins).__name__.replace("Inst", "", 1)
                if (
                    si is not None
                    and si.on_wait
                    and opname not in skip
                    and len(si.on_wait) > caps.get(opname, 1)
                ):
                    cap = caps.get(opname, 1)
                    waits = list(si.on_wait)
                    excess, keep = waits[:-cap], waits[-cap:]
                    for k, w in enumerate(excess):
                        ev = mybir.InstEventSemaphore(
                            name=f"{ins.name}-wsp{k}",
                            ins=[],
                            outs=[],
                            sync_info=mybir.SyncInfo(on_wait=[w], on_update=[]),
                        )
                        ev.engine = ins.engine
                        out.append(ev)
                        n_split += 1
                    ins.sync_info = mybir.SyncInfo(
                        on_wait=keep, on_update=list(si.on_update)
                    )
                    changed = True
                out.append(ins)
            if changed:
                blk.instructions = out
    return n_split


def _get_nc():
    if "nc" not in _cache:
        _cache["nc"] = build_nc()
    return _cache["nc"]


def _pack_inputs(x, hx, cx, W, b, Wm, bm, gammas, betas):
    """Host-side packing: bf16 casts + transposed tile layouts."""
    import ml_dtypes

    bf = ml_dtypes.bfloat16
    xb = np.asarray(x, np.float32).astype(bf)
    hxb = np.asarray(hx, np.float32).astype(bf)
    cxf = np.ascontiguousarray(np.asarray(cx, np.float32))
    # W[kb*128+p, c*512+n] -> wp4[p, c, kb, n]
    wp4 = np.ascontiguousarray(
        np.asarray(W, np.float32)
        .astype(bf)
        .reshape(NKB2, P, NCHUNK, CHUNK)
        .transpose(1, 2, 0, 3)
    )
    # Wm[kb*128+p, n] -> wmp[p, kb, n]
    wmp = np.ascontiguousarray(
        np.asarray(Wm, np.float32).astype(bf).reshape(NKB1, P, DIM_H).transpose(1, 0, 2)
    )
    shared = {
        "wp4": wp4,
        "wmp": wmp,
        "bb": np.ascontiguousarray(np.asarray(b, np.float32).astype(bf)),
        "bmb": np.ascontiguousarray(np.asarray(bm, np.float32).astype(bf)),
        "gmb": np.ascontiguousarray(np.asarray(gammas, np.float32).astype(bf)),
        "btb": np.ascontiguousarray(np.asarray(betas, np.float32).astype(bf)),
    }
    in_maps = []
    for i in range(NCORES):
        sl = slice(i * BL, (i + 1) * BL)
        xc, hc = xb[sl], hxb[sl]
        # x[t*128+r, kb*128+p] -> xt4[p, t, kb, r]
        xt4 = np.ascontiguousarray(
            xc.reshape(NBT, P, NKB1, P).transpose(3, 0, 2, 1)
        )
        ht4 = np.ascontiguousarray(
            hc.reshape(NBT, P, NKB1, P).transpose(3, 0, 2, 1)
        )
        in_maps.append(
            {
                "xt4": xt4,
                "ht4": ht4,
                "hxr": np.ascontiguousarray(hc),
                "cxr": cxf[sl],
                **shared,
            }
        )
    return in_maps


def kernel(x, hx, cx, W, b, Wm, bm, gammas, betas):
    from concourse.bass_utils import run_bass_kernel_spmd

    nc = _get_nc()
    in_maps = _pack_inputs(x, hx, cx, W, b, Wm, bm, gammas, betas)
    res = run_bass_kernel_spmd(nc, in_maps, list(range(NCORES)))
    hx_mod = np.concatenate([r["hx_out"] for r in res.results], axis=0)
    cx_new = np.concatenate([r["cx_out"] for r in res.results], axis=0)
    return (hx_mod, cx_new)


# revision 38
# speedup vs baseline: 1.0770x; 1.0144x over previous
"""Trainium2 Bass kernel for the cosine-gated LSTM cell (CGLSTMCellv1).

Full inputs in, full outputs out. Data-parallel: batch sharded across 8
NeuronCores, weights replicated, no cross-core comms.

Key performance choices vs the fp32 baseline:
  - All matmuls in bf16 (fp32 matmul runs at 4 cycles/row on TRN2 PE; bf16
    at 1). Operands are cast and laid out host-side.
  - x^T / hx^T are packed on the host into the exact SBUF tile layout
    [p, t, kb, r], eliminating all 128 PE transposes + PSUM round-trips.
  - W is host-packed per column-chunk [p, c, kb, n] so each chunk load is
    one contiguous 16KB-per-partition DMA; chunks are triple-buffered.
  - DMAs are issued in first-use order (Wm + x^T tiles first) so the first
    matmul starts ~10us in, not after all input DMAs.
  - LayerNorm rstd and the two cosine denominators use a DVE-only
    Newton-iteration rsqrt (bit hack + 2 iterations) instead of ACT Sqrt,
    so the scalar engine needs exactly one activation-table set
    (sigmoid_and_others: sigmoid/tanh/square/copy) -> one table load total.
  - Gate bias b is added during the PSUM->SBUF combine pass (free).
  - LN mean comes free from the combine pass accum; E[z^2] from an ACT
    Square accum pass; stats are batched per half-wave (4 tiles) on [P,4]
    tiles to shorten the stats barrier.
  - The o-gate's normalize/affine work runs on DVE (not Pool) because it
    sits in the post-matmul tail where Pool's 2.3us/op latency would be
    exposed; the earlier gates' affine runs on Pool to offload DVE while
    the PE still has matmul work to overlap.

Math per core (rows = local batch slice):
  mapped = x @ Wm + bm
  attn   = sigmoid(cos_sim(mapped, hx));  s = 1 + attn
  gates  = concat(s*x, hx) @ W + b   (s folded into x^T)
  i,f,g,o = LN-gates -> sigmoid/tanh
  cx_new = f*cx + i*g ; hx_new = o*tanh(cx_new)
  hx_mod = hx_new * (1 + sigmoid((cos_sim(hx_new,cx_new)+1)/2))
"""

import numpy as np

B_FULL, DIM_I, DIM_H = 8192, 1024, 1024
NCORES = 8
BL = B_FULL // NCORES  # 1024 rows per core
P = 128
H4 = 4 * DIM_H
NBT = BL // P               # 8 row tiles of 128
NKB1 = DIM_I // P           # 8  k-blocks for the x part
NKB2 = (DIM_I + DIM_H) // P  # 16 k-blocks for mm2
CHUNK = 512                 # W column chunk (= one PSUM bank of fp32)
NCHUNK = H4 // CHUNK        # 8 chunks total, 2 per gate
LN_EPS = 1e-5
COS_EPS2 = 1e-12
MAGIC = 0x5F3759DF

_cache = {}


def build_nc(nbt=NBT):
    from contextlib import ExitStack

    import concourse.bass as bass
    import concourse.mybir as mybir
    import concourse.tile as tile

    fp32 = mybir.dt.float32
    bf16 = mybir.dt.bfloat16
    i32 = mybir.dt.int32
    AF = mybir.ActivationFunctionType
    OP = mybir.AluOpType
    bl = nbt * P

    nc = bass.Bass()
    # host-packed inputs (see kernel() for the packing)
    xtd = nc.dram_tensor("xt4", [P, nbt, NKB1, P], bf16, kind="ExternalInput")
    htd = nc.dram_tensor("ht4", [P, nbt, NKB1, P], bf16, kind="ExternalInput")
    hxd = nc.dram_tensor("hxr", [bl, DIM_H], bf16, kind="ExternalInput")
    cxd = nc.dram_tensor("cxr", [bl, DIM_H], fp32, kind="ExternalInput")
    wpd = nc.dram_tensor("wp4", [P, NCHUNK, NKB2, CHUNK], bf16, kind="ExternalInput")
    wmd = nc.dram_tensor("wmp", [P, NKB1, DIM_H], bf16, kind="ExternalInput")
    bd = nc.dram_tensor("bb", [H4], bf16, kind="ExternalInput")
    bmd = nc.dram_tensor("bmb", [DIM_H], bf16, kind="ExternalInput")
    gd = nc.dram_tensor("gmb", [4, DIM_H], bf16, kind="ExternalInput")
    btd = nc.dram_tensor("btb", [4, DIM_H], bf16, kind="ExternalInput")
    hxo = nc.dram_tensor("hx_out", [bl, DIM_H], fp32, kind="ExternalOutput")
    cxo = nc.dram_tensor("cx_out", [bl, DIM_H], fp32, kind="ExternalOutput")

    def bcast_row(src_ap):
        # view an [N]-shaped AP as [P, N] with 0-step partition broadcast
        return bass.AP(
            tensor=src_ap.tensor, offset=src_ap.offset, ap=[[0, P]] + list(src_ap.ap)
        )

    with tile.TileContext(nc) as tc, ExitStack() as ctx:
        singles = ctx.enter_context(tc.tile_pool(name="singles", bufs=1))
        sm_pool = ctx.enter_context(tc.tile_pool(name="smalls", bufs=3))

        from concourse.masks import make_identity

        ident = singles.tile([P, P], fp32)
        ones128 = singles.tile([P, P], fp32)
        zrow = singles.tile([P, P], fp32)
        magic_t = singles.tile([P, 8], i32)
        halfc = singles.tile([P, 1], fp32)
        junk = singles.tile([P, 1], fp32)

        def emit_consts():
            make_identity(nc, ident)
            nc.vector.memset(ones128, 1.0)
            nc.vector.memset(zrow, 0.0)
            nc.vector.memset(magic_t, MAGIC)
            nc.vector.memset(halfc, 0.5)
            nc.vector.memset(junk, 0.5)
            # prime the ACT table with the one set the whole kernel uses
            nc.scalar.activation(junk, junk, AF.Sigmoid)

        def newton_rsqrt(x_f32, n, name):
            """y ~= 1/sqrt(x) on DVE only: bit hack + 2 Newton steps."""
            xi = x_f32.bitcast(i32)
            sh = sm_pool.tile([P, n], i32, tag="nr_sh", name=f"sh_{name}")
            nc.vector.tensor_scalar(sh, xi, 1, None, op0=OP.logical_shift_right)
            yi = sm_pool.tile([P, n], i32, tag="nr_yi", name=f"yi_{name}")
            nc.vector.tensor_tensor(yi, magic_t[:, 0:n], sh, OP.subtract)
            y = yi.bitcast(fp32)
            for it in range(2):
                a = sm_pool.tile([P, n], fp32, tag="nr_a", name=f"a{it}_{name}")
                nc.vector.tensor_tensor(a, y, y, OP.mult)
                nc.vector.tensor_tensor(a, a, x_f32, OP.mult)
                nc.vector.tensor_scalar(a, a, -0.5, 1.5, OP.mult, OP.add)
                yn = sm_pool.tile([P, n], fp32, tag="nr_y", name=f"y{it}_{name}")
                nc.vector.tensor_tensor(yn, y, a, OP.mult)
                y = yn
            return y

        # persistent transposed activations (xsT scaled in place after mm1)
        xsT = singles.tile([P, nbt, NKB1, P], bf16)
        hxT = singles.tile([P, nbt, NKB1, P], bf16)
        bm_rep = singles.tile([P, DIM_H], bf16)

        # cos-gate scratch, batched across tiles as [P, nbt] columns
        dot_a = sm_pool.tile([P, nbt], fp32, tag="dot_a")
        sqm_a = sm_pool.tile([P, nbt], fp32, tag="sqm_a")
        sqh_a = sm_pool.tile([P, nbt], fp32, tag="sqh_a")
        s_a = sm_pool.tile([P, nbt], fp32, tag="s_a")

        iact_pool = ctx.enter_context(tc.tile_pool(name="iact", bufs=nbt))
        iact = [
            iact_pool.tile([P, DIM_H], fp32, tag="iact", name=f"iact{t}")
            for t in range(nbt)
        ]
        w_pool = ctx.enter_context(tc.tile_pool(name="wch", bufs=2))
        dump_pool = ctx.enter_context(tc.tile_pool(name="dump", bufs=3))
        ps_rep = ctx.enter_context(tc.tile_pool(name="psrep", bufs=2, space="PSUM"))
        wch = {}

        def load_wch(cid):
            if cid in wch or cid >= NCHUNK:
                return
            wch[cid] = w_pool.tile(
                [P, NKB2, CHUNK], bf16, tag="wch", name=f"wch{cid}"
            )
            nc.sync.dma_start(out=wch[cid], in_=wpd[:, cid])

        # s-fold machinery: replicate s across partitions via PE, then
        # scale x^T in place. Emitted at points where the PE would
        # otherwise be busy so the in-order PE stream never stalls on it.
        def fold_s(tiles):
            for tt in tiles:
                psT = ps_rep.tile([1, P], fp32, tag="rep", name=f"psT{tt}")
                nc.tensor.transpose(psT, s_a[:, tt : tt + 1], ident)
                # copies on DVE: ACT is saturated with squares in phase 1
                nc.vector.tensor_copy(zrow[0:1, :], psT)
                psr = ps_rep.tile([P, P], fp32, tag="rep", name=f"psr{tt}")
                nc.tensor.matmul(psr, ones128, zrow, start=True, stop=True)
                srep = sm_pool.tile([P, P], bf16, tag="srep", name=f"sr{tt}")
                nc.vector.tensor_copy(srep, psr)
                srep_brd = bass.AP(
                    tensor=srep.tensor,
                    offset=srep.offset,
                    ap=[list(srep.ap[0]), [0, NKB1], list(srep.ap[1])],
                )
                nc.vector.tensor_tensor(xsT[:, tt], xsT[:, tt], srep_brd, OP.mult)

        # ---------------- phase 1: mm1 + cosine gate ----------------
        with ExitStack() as p1:
            wm_pool = p1.enter_context(tc.tile_pool(name="wm", bufs=1))
            hx_pool = p1.enter_context(tc.tile_pool(name="hxr", bufs=1))
            map_pool = p1.enter_context(tc.tile_pool(name="map", bufs=3))
            ps_m1 = p1.enter_context(tc.tile_pool(name="psm1", bufs=2, space="PSUM"))

            # DMAs first, in first-use order, before any const setup: Wm and
            # x^T tiles gate the first matmuls; W chunks 0/1 and hx^T must
            # land before the first wave (~35us). Split the DGE configs
            # across SP and ACT so transfers start sooner.
            wm_sb = wm_pool.tile([P, NKB1, DIM_H], bf16)
            nc.sync.dma_start(out=wm_sb, in_=wmd[:])
            for t in range(nbt):
                nc.scalar.dma_start(out=xsT[:, t], in_=xtd[:, t])
            nc.sync.dma_start(out=bm_rep, in_=bcast_row(bmd[:]))
            load_wch(0)
            hx_all = hx_pool.tile([P, nbt, DIM_H], bf16)
            nc.sync.dma_start(
                out=hx_all, in_=hxd[:].rearrange("(t p) n -> p t n", p=P)
            )
            for t in range(nbt):
                nc.scalar.dma_start(out=hxT[:, t], in_=htd[:, t])
            load_wch(1)
            emit_consts()

            # ||hx||^2 per tile, early (ACT Square, accum)
            for t in range(nbt):
                dmp = dump_pool.tile([P, DIM_H], bf16, tag="dump", name=f"dhx{t}")
                nc.scalar.activation(
                    dmp, hx_all[:, t], AF.Square, accum_out=sqh_a[:, t : t + 1]
                )

            for t in range(nbt):
                # mapped = x @ Wm + bm   (2 chunks of 512)
                map_sb = map_pool.tile([P, DIM_H], fp32, tag="map", name=f"map{t}")
                for ch in range(2):
                    cs = slice(ch * CHUNK, (ch + 1) * CHUNK)
                    pm = ps_m1.tile([P, CHUNK], fp32, tag="pm", name=f"pm{t}_{ch}")
                    for kb in range(NKB1):
                        nc.tensor.matmul(
                            pm,
                            xsT[:, t, kb, :],
                            wm_sb[:, kb, cs],
                            start=(kb == 0),
                            stop=(kb == NKB1 - 1),
                        )
                    nc.vector.tensor_tensor(map_sb[:, cs], pm, bm_rep[:, cs], OP.add)
                # dot(mapped, hx) via STT with accumulate
                dmp = dump_pool.tile([P, DIM_H], bf16, tag="dump", name=f"ddot{t}")
                nc.vector.scalar_tensor_tensor(
                    out=dmp,
                    in0=map_sb,
                    scalar=1.0,
                    in1=hx_all[:, t],
                    op0=OP.mult,
                    op1=OP.mult,
                    accum_out=dot_a[:, t : t + 1],
                )
                # ||mapped||^2
                dmp2 = dump_pool.tile([P, DIM_H], bf16, tag="dump", name=f"dsq{t}")
                nc.scalar.activation(
                    dmp2, map_sb, AF.Square, accum_out=sqm_a[:, t : t + 1]
                )

                if t % 4 == 3:
                    # batched cosine finish for tiles t-3..t
                    g0 = t - 3
                    bs = slice(g0, t + 1)
                    m1 = sm_pool.tile([P, 4], fp32, tag="c_m1", name=f"m1_{t}")
                    nc.vector.tensor_scalar_max(m1, sqm_a[:, bs], COS_EPS2)
                    m2 = sm_pool.tile([P, 4], fp32, tag="c_m2", name=f"m2_{t}")
                    nc.vector.tensor_scalar_max(m2, sqh_a[:, bs], COS_EPS2)
                    den = sm_pool.tile([P, 4], fp32, tag="c_den", name=f"den_{t}")
                    nc.vector.tensor_tensor(den, m1, m2, OP.mult)
                    rinv = newton_rsqrt(den, 4, f"cos{t}")
                    cosv = sm_pool.tile([P, 4], fp32, tag="c_cos", name=f"cos_{t}")
                    nc.vector.tensor_tensor(cosv, dot_a[:, bs], rinv, OP.mult)
                    att = sm_pool.tile([P, 4], fp32, tag="c_att", name=f"att_{t}")
                    nc.scalar.activation(att, cosv, AF.Sigmoid)
                    nc.vector.tensor_scalar_add(s_a[:, bs], att, 1.0)
                if t == 5:
                    # batch-A fold lands while mm1 t6/t7 keeps the PE busy
                    fold_s(range(0, 4))

        # ---------------- phase 2: gates ----------------
        # two alternating z pools so a wave's combines never wait on the
        # previous wave's apply passes to release slots
        z_pools = [
            ctx.enter_context(tc.tile_pool(name="z0", bufs=nbt)),
            ctx.enter_context(tc.tile_pool(name="z1", bufs=nbt)),
        ]
        gb_pool = ctx.enter_context(tc.tile_pool(name="gb", bufs=2))
        wk_pool = ctx.enter_context(tc.tile_pool(name="wk", bufs=2))
        ga_pool = ctx.enter_context(tc.tile_pool(name="ga", bufs=2))
        cx_pool = ctx.enter_context(tc.tile_pool(name="cxin", bufs=2))
        hxn_pool = ctx.enter_context(tc.tile_pool(name="hxn", bufs=nbt))
        ps_g = ctx.enter_context(tc.tile_pool(name="psg", bufs=6, space="PSUM"))

        def wave_tiles(name):
            return sm_pool.tile([P, nbt], fp32, tag=f"wv_{name}", name=name)

        # ||cx_new||^2 is produced during the f wave; tanh(cx_new) tiles are
        # produced at the top of the o wave (both depend only on f output)
        s2w = wave_tiles("o_s2")
        tnhs = {}

        wave_cids = {0: (0, 1), 2: (4, 5), 1: (2, 3), 3: (6, 7)}
        gate_order = (
            (0, AF.Sigmoid, "i"),
            (2, AF.Tanh, "g"),
            (1, AF.Sigmoid, "f"),
            (3, AF.Sigmoid, "o"),
        )

        for wi, (gi, func, role) in enumerate(gate_order):
            cids = wave_cids[gi]
            for cid in cids:
                load_wch(cid)
            # prefetch next wave's first chunk (3rd buffer)
            if wi + 1 < 4:
                load_wch(wave_cids[gate_order[wi + 1][0]][0])

            grep = gb_pool.tile([P, DIM_H], bf16, tag="grep", name=f"grep{gi}")
            nc.sync.dma_start(out=grep, in_=bcast_row(gd[gi, :]))
            brep = gb_pool.tile([P, DIM_H], bf16, tag="brep", name=f"brep{gi}")
            nc.sync.dma_start(out=brep, in_=bcast_row(btd[gi, :]))
            brow = gb_pool.tile([P, DIM_H], bf16, tag="brow", name=f"brow{gi}")
            nc.sync.dma_start(
                out=brow, in_=bcast_row(bd[gi * DIM_H : (gi + 1) * DIM_H])
            )

            if role == "o":
                for t in range(nbt):
                    tnh = hxn_pool.tile(
                        [P, DIM_H], fp32, tag="tnh", name=f"tnh{t}"
                    )
                    nc.scalar.activation(tnh, iact[t], AF.Tanh)
                    tnhs[t] = tnh

            zs = [None] * nbt
            zsum = wave_tiles(f"zsum{gi}")  # per-chunk row sums
            zsum2 = wave_tiles(f"zsum2{gi}")
            zss = wave_tiles(f"zss{gi}")  # row sums of squares
            mean = wave_tiles(f"mean{gi}")
            msq = wave_tiles(f"msq{gi}")
            vare = wave_tiles(f"vare{gi}")

            if role == "o":
                d2w = wave_tiles("o_d2")
                s1w = wave_tiles("o_s1")
                hxns = {}

            for half in range(2):
                tl = range(half * (nbt // 2), (half + 1) * (nbt // 2))
                hs = slice(half * (nbt // 2), (half + 1) * (nbt // 2))
                if wi == 0 and half == 1:
                    # batch-B s-fold: the PE is busy with wave-0 half-0
                    # matmuls while the batch-B cosine chain completes
                    fold_s(range(4, 8))
                for c, cid in enumerate(cids):
                    col0 = cid * CHUNK
                    for t in tl:
                        ps = ps_g.tile(
                            [P, CHUNK], fp32, tag="pg", name=f"pg{cid}_{t}"
                        )
                        for kb in range(NKB2):
                            lhsT = (
                                xsT[:, t, kb, :]
                                if kb < NKB1
                                else hxT[:, t, kb - NKB1, :]
                            )
                            nc.tensor.matmul(
                                ps,
                                lhsT,
                                wch[cid][:, kb, :],
                                start=(kb == 0),
                                stop=(kb == NKB2 - 1),
                            )
                        if c == 0:
                            zs[t] = z_pools[wi % 2].tile(
                                [P, DIM_H], bf16, tag="z", name=f"z{gi}_{t}"
                            )
                        # combine: z = ps + b (accumulate row-sum for the mean)
                        acc = zsum if c == 0 else zsum2
                        nc.vector.scalar_tensor_tensor(
                            out=zs[t][:, c * CHUNK : (c + 1) * CHUNK],
                            in0=ps,
                            scalar=0.0,
                            in1=brow[:, c * CHUNK : (c + 1) * CHUNK],
                            op0=OP.add,
                            op1=OP.add,
                            accum_out=acc[:, t : t + 1],
                        )

                # E[z^2] per tile via ACT Square accum
                for t in tl:
                    dmp = dump_pool.tile(
                        [P, DIM_H], bf16, tag="dump", name=f"dz{gi}{t}"
                    )
                    nc.scalar.activation(
                        dmp, zs[t], AF.Square, accum_out=zss[:, t : t + 1]
                    )

                # batched LN stats for this half on [P,4] slices
                nc.vector.tensor_tensor(mean[:, hs], zsum[:, hs], zsum2[:, hs], OP.add)
                nc.vector.tensor_scalar_mul(mean[:, hs], mean[:, hs], 1.0 / DIM_H)
                nc.vector.tensor_tensor(msq[:, hs], mean[:, hs], mean[:, hs], OP.mult)
                nc.vector.scalar_tensor_tensor(
                    out=vare[:, hs],
                    in0=zss[:, hs],
                    scalar=1.0 / DIM_H,
                    in1=msq[:, hs],
                    op0=OP.mult,
                    op1=OP.subtract,
                )
                nc.vector.tensor_scalar_add(vare[:, hs], vare[:, hs], LN_EPS)
                rstd = newton_rsqrt(vare[:, hs], nbt // 2, f"ln{gi}h{half}")

                for j, t in enumerate(tl):
                    if role == "i":
                        ga = iact[t]
                    else:
                        ga = ga_pool.tile(
                            [P, DIM_H], fp32, tag="ga", name=f"ga{gi}_{t}"
                        )
                    if role == "o":
                        # tail gate: keep everything on DVE (Pool latency
                        # would sit exposed after the last matmuls)
                        wk = wk_pool.tile(
                            [P, DIM_H], bf16, tag="wk", name=f"wk{gi}_{t}"
                        )
                        nc.vector.scalar_tensor_tensor(
                            out=wk,
                            in0=zs[t],
                            scalar=mean[:, t : t + 1],
                            in1=grep,
                            op0=OP.subtract,
                            op1=OP.mult,
                        )
                        nc.vector.scalar_tensor_tensor(
                            out=ga,
                            in0=wk,
                            scalar=rstd[:, j : j + 1],
                            in1=brep,
                            op0=OP.mult,
                            op1=OP.add,
                        )
                    else:
                        # u = (z - mean) * rstd on DVE, affine on Pool
                        wk = wk_pool.tile(
                            [P, DIM_H], bf16, tag="wk", name=f"wk{gi}_{t}"
                        )
                        nc.vector.tensor_scalar(
                            wk,
                            zs[t],
                            mean[:, t : t + 1],
                            rstd[:, j : j + 1],
                            OP.subtract,
                            OP.mult,
                        )
                        nc.gpsimd.tensor_tensor(ga, wk, grep, OP.mult)
                        nc.gpsimd.tensor_tensor(ga, ga, brep, OP.add)
                    nc.scalar.activation(ga, ga, func)

                    if role == "g":
                        nc.vector.tensor_tensor(iact[t], iact[t], ga, OP.mult)
                    elif role == "f":
                        # cx chain on DVE: Pool is saturated by the applies
                        # in this wave and would stall the o-wave matmuls
                        cx_t = cx_pool.tile(
                            [P, DIM_H], fp32, tag="cx", name=f"cx{t}"
                        )
                        nc.sync.dma_start(
                            out=cx_t, in_=cxd[t * P : (t + 1) * P, :]
                        )
                        nc.vector.tensor_tensor(cx_t, ga, cx_t, OP.mult)
                        nc.vector.tensor_tensor(iact[t], iact[t], cx_t, OP.add)
                        nc.scalar.dma_start(
                            out=cxo[t * P : (t + 1) * P, :], in_=iact[t]
                        )
                        # ||cx_new||^2 for the o-wave cosine, done here early
                        dmp = dump_pool.tile(
                            [P, DIM_H], bf16, tag="dump", name=f"dsq2{t}"
                        )
                        nc.scalar.activation(
                            dmp, iact[t], AF.Square, accum_out=s2w[:, t : t + 1]
                        )
                    elif role == "o":
                        # hx_new = o * tanh(cx_new); tanh precomputed
                        hxn = tnhs[t]
                        nc.gpsimd.tensor_tensor(hxn, ga, hxn, OP.mult)
                        hxns[t] = hxn
                        # second cosine gate: accumulate dot/norm per tile
                        dmp = dump_pool.tile(
                            [P, DIM_H], bf16, tag="dump", name=f"do{t}"
                        )
                        nc.vector.scalar_tensor_tensor(
                            out=dmp,
                            in0=hxn,
                            scalar=1.0,
                            in1=iact[t],
                            op0=OP.mult,
                            op1=OP.mult,
                            accum_out=d2w[:, t : t + 1],
                        )
                        dmpa = dump_pool.tile(
                            [P, DIM_H], bf16, tag="dump", name=f"da{t}"
                        )
                        nc.scalar.activation(
                            dmpa, hxn, AF.Square, accum_out=s1w[:, t : t + 1]
                        )

                if role == "o":
                    # batched second-cosine finish for this half's 4 tiles
                    ma = sm_pool.tile([P, 4], fp32, tag="o_ma", name=f"ma_{half}")
                    nc.vector.tensor_scalar_max(ma, s1w[:, hs], COS_EPS2)
                    mb = sm_pool.tile([P, 4], fp32, tag="o_mb", name=f"mb_{half}")
                    nc.vector.tensor_scalar_max(mb, s2w[:, hs], COS_EPS2)
                    dn2 = sm_pool.tile([P, 4], fp32, tag="o_dn", name=f"dn_{half}")
                    nc.vector.tensor_tensor(dn2, ma, mb, OP.mult)
                    rr2 = newton_rsqrt(dn2, 4, f"o{half}")
                    cos2 = sm_pool.tile([P, 4], fp32, tag="o_cs", name=f"cs_{half}")
                    nc.vector.tensor_tensor(cos2, d2w[:, hs], rr2, OP.mult)
                    co = sm_pool.tile([P, 4], fp32, tag="o_co", name=f"co_{half}")
                    # sigmoid((cos+1)/2) = sigmoid(0.5*cos + 0.5)
                    nc.scalar.activation(co, cos2, AF.Sigmoid, bias=halfc, scale=0.5)
                    cop = sm_pool.tile([P, 4], fp32, tag="o_cp", name=f"cp_{half}")
                    nc.vector.tensor_scalar_add(cop, co, 1.0)
                    for j, t in enumerate(tl):
                        # final scale on Pool: gpsimd can't take an AP scalar,
                        # so replicate (1+co) along the free dim via 0-step AP
                        cop_col = cop[:, j : j + 1]
                        cop_brd = bass.AP(
                            tensor=cop_col.tensor,
                            offset=cop_col.offset,
                            ap=[list(cop_col.ap[0]), [0, DIM_H]],
                        )
                        nc.gpsimd.tensor_tensor(hxns[t], hxns[t], cop_brd, OP.mult)
                        nc.scalar.dma_start(
                            out=hxo[t * P : (t + 1) * P, :], in_=hxns[t]
                        )
    _split_excess_waits(nc)
    return nc


def _split_excess_waits(nc):
    """Walrus ISA structs have limited sync-wait slots (Matmult/LDW: 1,
    DMA: 2, several DVE/ACT structs: 1-2). The Tile scheduler can emit more.
    Move excess waits onto standalone EventSemaphore instructions injected
    just before the offender on the same engine."""
    import concourse.mybir as mybir

    caps = {}
    skip = {"EventSemaphore", "RegisterMove", "UnconditionalBranch"}
    n_split = 0
    for fn in nc.m.functions:
        for blk in fn.blocks:
            out = []
            changed = False
            for ins in blk.instructions:
                si = ins.sync_info
                opname = type(ins).__name__.replace("Inst", "", 1)
                if (
                    si is not None
                    and si.on_wait
                    and opname not in skip
                    and len(si.on_wait) > caps.get(opname, 1)
                ):
                    cap = caps.get(opname, 1)
                    waits = list(si.on_wait)
                    excess, keep = waits[:-cap], waits[-cap:]
                    for k, w in enumerate(excess):
                        ev = mybir.InstEventSemaphore(
                            name=f"{ins.name}-wsp{k}",
                            ins=[],
                            outs=[],
                            sync_info=mybir.SyncInfo(on_wait=[w], on_update=[]),
                        )
                        ev.engine = ins.engine
                        out.append(ev)
                        n_split += 1
                    ins.sync_info = mybir.SyncInfo(
                        on_wait=keep, on_update=list(si.on_update)
                    )
                    changed = True
                out.append(ins)
            if changed:
                blk.instructions = out
    return n_split


def _get_nc():
    if "nc" not in _cache:
        _cache["nc"] = build_nc()
    return _cache["nc"]


def _pack_inputs(x, hx, cx, W, b, Wm, bm, gammas, betas):
    """Host-side packing: bf16 casts + transposed tile layouts."""
    import ml_dtypes

    bf = ml_dtypes.bfloat16
    xb = np.asarray(x, np.float32).astype(bf)
    hxb = np.asarray(hx, np.float32).astype(bf)
    cxf = np.ascontiguousarray(np.asarray(cx, np.float32))
    # W[kb*128+p, c*512+n] -> wp4[p, c, kb, n]
    wp4 = np.ascontiguousarray(
        np.asarray(W, np.float32)
        .astype(bf)
        .reshape(NKB2, P, NCHUNK, CHUNK)
        .transpose(1, 2, 0, 3)
    )
    # Wm[kb*128+p, n] -> wmp[p, kb, n]
    wmp = np.ascontiguousarray(
        np.asarray(Wm, np.float32).astype(bf).reshape(NKB1, P, DIM_H).transpose(1, 0, 2)
    )
    shared = {
        "wp4": wp4,
        "wmp": wmp,
        "bb": np.ascontiguousarray(np.asarray(b, np.float32).astype(bf)),
        "bmb": np.ascontiguousarray(np.asarray(bm, np.float32).astype(bf)),
        "gmb": np.ascontiguousarray(np.asarray(gammas, np.float32).astype(bf)),
        "btb": np.ascontiguousarray(np.asarray(betas, np.float32).astype(bf)),
    }
    in_maps = []
    for i in range(NCORES):
        sl = slice(i * BL, (i + 1) * BL)
        xc, hc = xb[sl], hxb[sl]
        # x[t*128+r, kb*128+p] -> xt4[p, t, kb, r]
        xt4 = np.ascontiguousarray(
            xc.reshape(NBT, P, NKB1, P).transpose(3, 0, 2, 1)
        )
        ht4 = np.ascontiguousarray(
            hc.reshape(NBT, P, NKB1, P).transpose(3, 0, 2, 1)
        )
        in_maps.append(
            {
                "xt4": xt4,
                "ht4": ht4,
                "hxr": np.ascontiguousarray(hc),
                "cxr": cxf[sl],
                **shared,
            }
        )
    return in_maps


def kernel(x, hx, cx, W, b, Wm, bm, gammas, betas):
    from concourse.bass_utils import run_bass_kernel_spmd

    nc = _get_nc()
    in_maps = _pack_inputs(x, hx, cx, W, b, Wm, bm, gammas, betas)
    res = run_bass_kernel_spmd(nc, in_maps, list(range(NCORES)))
    hx_mod = np.concatenate([r["hx_out"] for r in res.results], axis=0)
    cx_new = np.concatenate([r["cx_out"] for r in res.results], axis=0)
    return (hx_mod, cx_new)
